# revision 24
# baseline (speedup 1.0000x reference)
"""Fused tensor-parallel transformer layer for Trainium2 (8 NeuronCores).

Sharding: Megatron-style tensor parallel. Each core owns 4 heads of the
attention block (q/k/v projection rows, o_proj columns) and 1/8 of the FFN
hidden dim (w1 rows, w2 columns). LayerNorms are computed replicated on
every core. One on-device AllReduce (bf16, Shared output) joins the
attention block to the FFN block; the final residual sum is assembled on
the host from per-core partial outputs (each core adds x2/8 so the
partials sum to the answer).

v2 layout: all matmul operands are bf16 (fast weight load + half the HBM
traffic), weights are DMA'd in one batched transfer per output tile,
K/V stay resident in SBUF (no DRAM round trip), the causal mask is a 0/1
multiply on the vector engine, and DMA issue is spread across the sync /
scalar / gpsimd queues. Activations stay transposed ([hid, seq]) so every
matmul contracts over the partition dim with zero on-device transposes.
"""

import math
import ml_dtypes
import numpy as np

import concourse.bass as bass
import concourse.mybir as mybir
import concourse.tile as tile
from concourse import bacc
from concourse.bass_utils import run_bass_kernel_spmd

FP = mybir.dt.float32
BF = mybir.dt.bfloat16
P = 128
EPS = 1e-6
AF = mybir.ActivationFunctionType
ALU = mybir.AluOpType
BF_NP = ml_dtypes.bfloat16


def fr(ap):
    return ap.bitcast(mybir.dt.float32r)


CFG_FULL = dict(
    seq=2048, hid=4096, ffn=16384, n_cores=8, n_heads=32,
    d_nope=128, d_rope=64, d_v=128, sb=512, ss=1024, fb=1024,
)


def build_layer_kernel(cfg, mask_mode, ln1_affine, ln2_affine):
    """mask_mode: 'causal' (skip tiles above diag, 0/1-multiply diag tiles),
    'zero' (no mask at all), 'full' (additive mask everywhere)."""
    seq, hid, ffn = cfg["seq"], cfg["hid"], cfg["ffn"]
    n_cores, n_heads = cfg["n_cores"], cfg["n_heads"]
    d_nope, d_rope, d_v = cfg["d_nope"], cfg["d_rope"], cfg["d_v"]
    SB, SS, FB = cfg["sb"], cfg["ss"], cfg["fb"]
    half = d_rope // 2
    hpc = n_heads // n_cores              # heads per core
    nkt = hid // P                        # hid k-tiles
    nsb = seq // SB                       # 512-wide blocks (attn q / stage E)
    sbt = SB // P                         # sk tiles per 512 block
    nskt = seq // P                       # total sk tiles
    nss = seq // SS                       # projection super blocks
    psub = SS // SB                       # 512 sub blocks per super block
    n_rope_ot = hpc * d_rope // P         # rope o-tiles (2 heads each)
    qo = hpc * d_nope // P + n_rope_ot    # q/k o-tiles per core
    dvc = hpc * d_v                       # v cols per core
    ndvt = dvc // P                       # o_proj contraction tiles
    fpc = ffn // n_cores                  # ffn rows per core
    nft = fpc // P                        # f tiles per core
    nfb = seq // FB                       # ffn s-blocks
    fsub = FB // SB                       # 512 sub blocks per ffn block
    assert hpc % 2 == 0 and half == 32 and d_nope == P and d_v == P
    assert nfb == 2 and nss == 2 and psub == 2 and fsub == 2

    nc = bacc.Bacc(None, target_bir_lowering=False)

    xt_d = nc.dram_tensor("xt", [nkt, P, seq], BF, kind="ExternalInput")
    wqk_d = nc.dram_tensor("wqk_t", [2 * qo, P, nkt * P], BF, kind="ExternalInput")
    wv_d = nc.dram_tensor("wv_t", [nkt, P, dvc], BF, kind="ExternalInput")
    wo_d = nc.dram_tensor("wo_t", [nkt, P, ndvt * P], BF, kind="ExternalInput")
    w1_d = nc.dram_tensor("w1_t", [nft, P, nkt * P], BF, kind="ExternalInput")
    w2_d = nc.dram_tensor("w2_t", [nkt, P, nft * P], BF, kind="ExternalInput")
    cos_d = nc.dram_tensor("cos_t", [P, seq], BF, kind="ExternalInput")
    sin_d = nc.dram_tensor("sin_t", [P, seq], BF, kind="ExternalInput")
    rp_d = nc.dram_tensor("rperm", [P, P], BF, kind="ExternalInput")
    onc_d = nc.dram_tensor("onc", [P, 1], BF, kind="ExternalInput")
    onr_d = nc.dram_tensor("onr", [1, P], FP, kind="ExternalInput")
    if mask_mode == "causal":
        mask_d = nc.dram_tensor("mask_t", [nsb, P, sbt * SB], BF, kind="ExternalInput")
    elif mask_mode == "full":
        mask_d = nc.dram_tensor("mask_t", [nskt, nsb, P, SB], FP, kind="ExternalInput")
    else:
        mask_d = None
    ln1_d = nc.dram_tensor("ln1_wb", [P, 2 * nkt], FP, kind="ExternalInput") if ln1_affine else None
    ln2_d = nc.dram_tensor("ln2_wb", [P, 2 * nkt], FP, kind="ExternalInput") if ln2_affine else None

    ar_in = nc.dram_tensor("ar_in", [nsb, nkt, P, SB], BF)
    ar_out = nc.dram_tensor("ar_out", [nsb, nkt, P, SB], BF, addr_space="Shared")
    x2_d = nc.dram_tensor("x2t", [nsb, nkt, P, SB], BF)
    y_d = nc.dram_tensor("y_t", [nkt, P, seq], FP, kind="ExternalOutput")

    q_nope_scale = 1.0 / math.sqrt(d_nope)
    q_rope_scale = 1.0 / math.sqrt(d_rope)

    _lp = nc.allow_low_precision(
        reason="bf16 matmul operands; fp32 SBUF views bitcast to float32r")
    _lp.__enter__()
    with tile.TileContext(nc) as tc:
        with (
            tc.tile_pool(name="const", bufs=1) as constp,
            tc.tile_pool(name="stat2", bufs=1) as stat2p,
            tc.tile_pool(name="stats", bufs=1) as statp,
            tc.tile_pool(name="sqp", bufs=2) as sqp,
            tc.tile_pool(name="xep", bufs=2) as xep,
            tc.tile_pool(name="psmm", bufs=7, space="PSUM") as psmm,
            tc.tile_pool(name="psln", bufs=1, space="PSUM") as psln,
        ):
            ones_col = constp.tile([P, 1], BF)
            nc.sync.dma_start(ones_col[:], onc_d[:, :])
            ones_row = constp.tile([1, P], FP)
            nc.sync.dma_start(fr(ones_row[:]), fr(onr_d[:, :]))
            ones_row_bf = constp.tile([1, P], BF)
            nc.vector.tensor_copy(out=ones_row_bf[:], in_=ones_row[:])
            eps_t = constp.tile([1, 1], FP)
            nc.any.memset(eps_t[:], EPS)
            rperm_t = constp.tile([P, P], BF)
            nc.sync.dma_start(rperm_t[:], rp_d[:, :])
            if ln1_affine:
                ln1_wb = constp.tile([P, 2 * nkt], FP)
                nc.sync.dma_start(ln1_wb[:], ln1_d[:, :])
            if ln2_affine:
                ln2_wb = constp.tile([P, 2 * nkt], FP)
                nc.sync.dma_start(ln2_wb[:], ln2_d[:, :])
            a2_all = stat2p.tile([1, seq], BF, tag="a2")
            c2_all = stat2p.tile([1, seq], BF, tag="c2")

            def bcast(row_sbuf):
                """[1, n<=SB] sbuf row -> [P, n] psum via PE rank-1 matmul."""
                n = row_sbuf.shape[-1]
                ps = psmm.tile([P, SB], FP, tag="mm", name="bc")
                ps = ps[:, :n]
                if row_sbuf.dtype == BF:
                    nc.tensor.matmul(ps, ones_row_bf[:], row_sbuf, start=True, stop=True)
                else:
                    nc.tensor.matmul(ps, fr(ones_row[:]), fr(row_sbuf), start=True, stop=True)
                return ps

            def ln_stats(sum_ps, ssq_ps):
                mu = statp.tile([1, SB], FP, tag="mu")
                nc.scalar.activation(mu[:], sum_ps[:1, :], AF.Copy, scale=1.0 / hid)
                msq = statp.tile([1, SB], FP, tag="msq")
                nc.scalar.activation(msq[:], ssq_ps[:1, :], AF.Copy, scale=1.0 / hid)
                var = statp.tile([1, SB], FP, tag="var")
                nc.vector.tensor_tensor(var[:], mu[:], mu[:], ALU.mult)
                nc.vector.tensor_tensor(var[:], msq[:], var[:], ALU.subtract)
                std = statp.tile([1, SB], FP, tag="std")
                nc.scalar.activation(std[:], var[:], AF.Sqrt, bias=eps_t[:])
                rstd = statp.tile([1, SB], FP, tag="rstd")
                nc.vector.reciprocal(fr(rstd[:]), std[:])
                nmr = statp.tile([1, SB], FP, tag="nmr")
                nc.vector.tensor_tensor(fr(nmr[:]), mu[:], rstd[:], ALU.mult)
                nc.vector.tensor_scalar_mul(fr(nmr[:]), nmr[:], -1.0)
                return rstd, nmr

            KC = 4                               # kt-chunk for batched DMA

            def stage_e(sb):
                """x2 = x + attn_allreduce; write x2 (bf16); LN2 stats."""
                ssl = slice(sb * SB, (sb + 1) * SB)
                pair_ps = psln.tile([P, SB], FP, tag="lnp")
                for kc in range(nkt // KC):
                    ksl = slice(kc * KC, (kc + 1) * KC)
                    xe = xep.tile([P, KC, SB], BF, tag="xe", bufs=2)
                    nc.sync.dma_start(xe[:], xt_d[ksl, :, ssl].transpose([1, 0, 2]))
                    are = xep.tile([P, KC, SB], BF, tag="are", bufs=2)
                    nc.gpsimd.dma_start(are[:], ar_out[sb, ksl].transpose([1, 0, 2]))
                    x2t = xep.tile([P, KC, SB], BF, tag="x2w", bufs=2)
                    for j in range(KC):
                        kt = kc * KC + j
                        nc.vector.tensor_tensor(x2t[:, j, :], xe[:, j, :], are[:, j, :], ALU.add)
                        sq = sqp.tile([P, SB], BF, tag="sq")
                        nc.vector.tensor_tensor(sq[:], x2t[:, j, :], x2t[:, j, :], ALU.mult)
                        nc.tensor.matmul(pair_ps[0:1, :], ones_col[:], x2t[:, j, :],
                                         start=(kt == 0), stop=(kt == nkt - 1),
                                         tile_position=(0, 0), skip_group_check=True)
                        nc.tensor.matmul(pair_ps[32:33, :], ones_col[:], sq[:],
                                         start=(kt == 0), stop=(kt == nkt - 1),
                                         tile_position=(0, 32), skip_group_check=True)
                    nc.scalar.dma_start(x2_d[sb, ksl].transpose([1, 0, 2]), x2t[:])
                rstd, nmr = ln_stats(pair_ps[0:1, :], pair_ps[32:33, :])
                nc.vector.tensor_copy(out=a2_all[:, ssl], in_=rstd[:])
                nc.vector.tensor_copy(out=c2_all[:, ssl], in_=nmr[:])

            # ---------------- attention block ----------------
            with (
                tc.tile_pool(name="ht", bufs=1) as htp,
                tc.tile_pool(name="qt", bufs=1) as qtp,
                tc.tile_pool(name="kall", bufs=1) as kallp,
                tc.tile_pool(name="vall", bufs=1) as vallp,
                tc.tile_pool(name="ctxp", bufs=1) as ctxp,
                tc.tile_pool(name="expp", bufs=3) as expp,
                tc.tile_pool(name="wqkp", bufs=2) as wqkp,
                tc.tile_pool(name="wvp", bufs=4) as wvp,
                tc.tile_pool(name="wop", bufs=3) as wop,
                tc.tile_pool(name="trig", bufs=1) as trigp,
                tc.tile_pool(name="maskp", bufs=1) as mp,
                tc.tile_pool(name="miscp", bufs=2) as miscp,
            ):
                k_all = kallp.tile([P, qo, seq], BF, tag="kall")
                v_all = vallp.tile([P, nskt, dvc], BF, tag="vall")

                def rope_apply(dest, raw, cs, sn):
                    """dest(bf16)/raw(bf16): [P, SB]; rows per 64-block: x1|x2.
                    out = raw*cos + swap(raw)*sin_signed, swap via PE perm."""
                    ps_sw = psmm.tile([P, SB], FP, tag="mm", name="swp")
                    nc.tensor.matmul(ps_sw[:], rperm_t[:], raw[:], start=True, stop=True)
                    m1 = miscp.tile([P, SB], BF, tag="mtmp", name="m1")
                    m2 = miscp.tile([P, SB], BF, tag="mtmp", name="m2")
                    nc.vector.tensor_tensor(m1[:], raw[:], cs, ALU.mult)
                    nc.vector.tensor_tensor(m2[:], ps_sw[:], sn, ALU.mult)
                    nc.vector.tensor_tensor(dest, m1[:], m2[:], ALU.add)

                def stage_ab(blk, qt):
                    """Load x for one 512-block, LN1 stats+normalize, q/k/v proj."""
                    bsl = slice(blk * SB, (blk + 1) * SB)
                    qloc = blk % psub
                    ht = htp.tile([P, nkt, SB], BF, tag="ht")
                    for kc in range(nkt // 8):
                        nc.sync.dma_start(
                            ht[:, kc * 8:(kc + 1) * 8, :],
                            xt_d[kc * 8:(kc + 1) * 8, :, bsl].transpose([1, 0, 2]))
                    pair_ps = psln.tile([P, SB], FP, tag="lnp")
                    for kt in range(nkt):
                        sq = sqp.tile([P, SB], BF, tag="sq")
                        nc.vector.tensor_tensor(sq[:], ht[:, kt, :], ht[:, kt, :], ALU.mult)
                        nc.tensor.matmul(pair_ps[0:1, :], ones_col[:], ht[:, kt, :],
                                         start=(kt == 0), stop=(kt == nkt - 1),
                                         tile_position=(0, 0), skip_group_check=True)
                        nc.tensor.matmul(pair_ps[32:33, :], ones_col[:], sq[:],
                                         start=(kt == 0), stop=(kt == nkt - 1),
                                         tile_position=(0, 32), skip_group_check=True)
                    rstd, nmr = ln_stats(pair_ps[0:1, :], pair_ps[32:33, :])
                    ab_ps = bcast(rstd[:])
                    cb_ps = bcast(nmr[:])
                    ab_bf = miscp.tile([P, SB], BF, tag="raw", name="ab_bf")
                    nc.scalar.activation(ab_bf[:], ab_ps[:], AF.Copy)
                    cb_bf = miscp.tile([P, SB], BF, tag="raw", name="cb_bf")
                    nc.scalar.activation(cb_bf[:], cb_ps[:], AF.Copy)
                    for kt in range(nkt):
                        nc.vector.tensor_tensor(ht[:, kt, :], ht[:, kt, :], ab_bf[:], ALU.mult)
                        nc.vector.tensor_tensor(ht[:, kt, :], ht[:, kt, :], cb_bf[:], ALU.add)
                        if ln1_affine:
                            nc.vector.tensor_scalar(
                                ht[:, kt, :], ht[:, kt, :],
                                ln1_wb[:, kt:kt + 1], ln1_wb[:, nkt + kt:nkt + kt + 1],
                                ALU.mult, ALU.add)

                    # --- q/k projections for this block ---
                    cs_b = trigp.tile([P, SB], BF, tag="cos", bufs=1)
                    nc.sync.dma_start(cs_b[:], cos_d[:, bsl])
                    sn_b = trigp.tile([P, SB], BF, tag="sin", bufs=1)
                    nc.sync.dma_start(sn_b[:], sin_d[:, bsl])
                    for ot in range(2 * qo):
                        whs = []
                        for hh in range(2):
                            wt = wqkp.tile([P, (nkt // 2) * P], BF, tag="wqk", bufs=2)
                            nc.sync.dma_start(
                                wt[:], wqk_d[ot, :, hh * (nkt // 2) * P:(hh + 1) * (nkt // 2) * P])
                            whs.append(wt)
                        ps = psmm.tile([P, SB], FP, tag="mm", name="proj")
                        for kt in range(nkt):
                            hh, kk = divmod(kt, nkt // 2)
                            nc.tensor.matmul(
                                ps[:], whs[hh][:, kk * P:(kk + 1) * P], ht[:, kt, :],
                                start=(kt == 0), stop=(kt == nkt - 1))
                        is_q = ot < qo
                        o = ot % qo
                        is_rope = o >= qo - n_rope_ot
                        if is_q:
                            dest = qt[:, o, qloc * SB:(qloc + 1) * SB]
                            scale = q_rope_scale if is_rope else q_nope_scale
                        else:
                            dest = k_all[:, o, bsl]
                            scale = 1.0
                        if not is_rope:
                            nc.scalar.activation(dest, ps[:], AF.Copy, scale=scale)
                        else:
                            raw = miscp.tile([P, SB], BF, tag="raw")
                            nc.scalar.activation(raw[:], ps[:], AF.Copy, scale=scale)
                            rope_apply(dest, raw[:], cs_b[:], sn_b[:])

                    # --- v projection for this block ---
                    v_pss = [psmm.tile([P, dvc], FP, tag="mm", name=f"vps{i}")
                             for i in range(sbt)]
                    for kc in range(nkt // KC):
                        wv4 = wvp.tile([P, KC, dvc], BF, tag="wv")
                        nc.sync.dma_start(
                            wv4[:], wv_d[kc * KC:(kc + 1) * KC].transpose([1, 0, 2]))
                        for j in range(KC):
                            kt = kc * KC + j
                            for sc in range(sbt):
                                nc.tensor.matmul(
                                    v_pss[sc][:], ht[:, kt, sc * P:(sc + 1) * P],
                                    wv4[:, j, :],
                                    start=(kt == 0), stop=(kt == nkt - 1))
                    for sc in range(sbt):
                        t_idx = blk * sbt + sc
                        nc.scalar.activation(v_all[:, t_idx, :], v_pss[sc][:], AF.Copy)

                def attn_superblock(ss):
                    qt = qtp.tile([P, qo, SS], BF, tag="qt")
                    for sub in range(psub):
                        stage_ab(ss * psub + sub, qt)

                    # --- stage C: both 512-blocks of this superblock together;
                    # shared k/v LDWEIGHTS, softmax sums col-tiled in one bank ---
                    sb0_, sb1_ = ss * psub, ss * psub + 1
                    t_max0 = (sb0_ + 1) * sbt if mask_mode == "causal" else nskt
                    t_max1 = (sb1_ + 1) * sbt if mask_mode == "causal" else nskt
                    if mask_mode == "causal":
                        mt0 = mp.tile([P, sbt * SB], BF, tag="mask")
                        nc.sync.dma_start(mt0[:], mask_d[sb0_])
                        mt1 = mp.tile([P, sbt * SB], BF, tag="mask1")
                        nc.sync.dma_start(mt1[:], mask_d[sb1_])
                    ctxt0 = ctxp.tile([P, hpc, SB], BF, tag="ctx")
                    ctxt1 = ctxp.tile([P, hpc, SB], BF, tag="ctx1")
                    for h in range(hpc):
                        rot = qo - n_rope_ot + h // 2
                        rsl = slice(64 * (h % 2), 64 * (h % 2) + 64)
                        sum_ps = psmm.tile([P, SB], FP, tag="mm", name="smx")
                        ctx_ps0 = psmm.tile([P, SB], FP, tag="mm", name="ctxps0")
                        ctx_ps1 = psmm.tile([P, SB], FP, tag="mm", name="ctxps1")
                        for t in range(t_max1):
                            tsl = slice(t * P, (t + 1) * P)
                            in0 = t < t_max0
                            if in0:
                                st0 = psmm.tile([P, SB], FP, tag="mm", name="st0")
                                nc.tensor.matmul(st0[:], k_all[:, h, tsl],
                                                 qt[:, h, 0:SB], start=True, stop=False)
                            st1 = psmm.tile([P, SB], FP, tag="mm", name="st1")
                            nc.tensor.matmul(st1[:], k_all[:, h, tsl],
                                             qt[:, h, SB:2 * SB], start=True, stop=False)
                            if in0:
                                nc.tensor.matmul(st0[:], k_all[rsl, rot, tsl],
                                                 qt[rsl, rot, 0:SB], start=False, stop=True)
                            nc.tensor.matmul(st1[:], k_all[rsl, rot, tsl],
                                             qt[rsl, rot, SB:2 * SB], start=False, stop=True)
                            if in0:
                                es0 = expp.tile([P, SB], BF, tag="es")
                                nc.scalar.activation(es0[:], st0[:], AF.Exp)
                                if mask_mode == "causal" and t >= sb0_ * sbt:
                                    i = t - sb0_ * sbt
                                    nc.vector.tensor_tensor(
                                        es0[:], es0[:], mt0[:, i * SB:(i + 1) * SB], ALU.mult)
                            es1 = expp.tile([P, SB], BF, tag="es")
                            nc.scalar.activation(es1[:], st1[:], AF.Exp)
                            if mask_mode == "causal" and t >= sb1_ * sbt:
                                i = t - sb1_ * sbt
                                nc.vector.tensor_tensor(
                                    es1[:], es1[:], mt1[:, i * SB:(i + 1) * SB], ALU.mult)
                            if in0:
                                nc.tensor.matmul(sum_ps[0:1, :], ones_col[:], es0[:],
                                                 start=(t == 0), stop=(t == t_max0 - 1),
                                                 tile_position=(0, 0), skip_group_check=True)
                            nc.tensor.matmul(sum_ps[32:33, :], ones_col[:], es1[:],
                                             start=(t == 0), stop=(t == t_max1 - 1),
                                             tile_position=(0, 32), skip_group_check=True)
                            if in0:
                                nc.tensor.matmul(ctx_ps0[:], v_all[:, t, h * P:(h + 1) * P],
                                                 es0[:], start=(t == 0), stop=(t == t_max0 - 1))
                            nc.tensor.matmul(ctx_ps1[:], v_all[:, t, h * P:(h + 1) * P],
                                             es1[:], start=(t == 0), stop=(t == t_max1 - 1))
                        for row, ctx_ps, ctxt in ((0, ctx_ps0, ctxt0), (32, ctx_ps1, ctxt1)):
                            sum_sb = statp.tile([1, SB], FP, tag="rec", bufs=1)
                            nc.scalar.activation(fr(sum_sb[:]), sum_ps[row:row + 1, :], AF.Copy)
                            rsum = statp.tile([1, SB], FP, tag="rsum", bufs=1)
                            nc.vector.reciprocal(fr(rsum[:]), sum_sb[:])
                            rb_ps = bcast(rsum[:])
                            rbw = miscp.tile([P, SB], BF, tag="ao", name="rbw")
                            nc.scalar.activation(rbw[:], rb_ps[:], AF.Copy)
                            nc.vector.tensor_tensor(ctxt[:, h, :], ctx_ps[:], rbw[:], ALU.mult)

                    # --- stage D: partial o_proj -> ar_in, per 512-block ---
                    for qb in range(psub):
                        sb = ss * psub + qb
                        ctxt = ctxt0 if qb == 0 else ctxt1
                        for hcc in range(nkt // KC):
                            wo4 = wop.tile([P, KC, ndvt * P], BF, tag="wo")
                            nc.sync.dma_start(
                                wo4[:], wo_d[hcc * KC:(hcc + 1) * KC].transpose([1, 0, 2]))
                            ao4 = miscp.tile([P, KC, SB], BF, tag="ao4")
                            for j in range(KC):
                                o_ps = psmm.tile([P, SB], FP, tag="mm", name="ops")
                                for dvt in range(ndvt):
                                    nc.tensor.matmul(o_ps[:], wo4[:, j, dvt * P:(dvt + 1) * P],
                                                     ctxt[:, dvt, :],
                                                     start=(dvt == 0), stop=(dvt == ndvt - 1))
                                nc.scalar.activation(ao4[:, j, :], o_ps[:], AF.Copy)
                            nc.scalar.dma_start(
                                ar_in[sb, hcc * KC:(hcc + 1) * KC].transpose([1, 0, 2]), ao4[:])
                        nc.gpsimd.collective_compute(
                            "AllReduce", ALU.add,
                            replica_groups=[list(range(n_cores))],
                            ins=[ar_in[sb].opt()], outs=[ar_out[sb].opt()])
                        if sb >= 2:
                            stage_e(sb - 2)

                for ss in range(nss):
                    attn_superblock(ss)

            # ---------------- FFN block ----------------
            with (
                tc.tile_pool(name="h2p", bufs=1) as h2p,
                tc.tile_pool(name="utp", bufs=1) as utp,
                tc.tile_pool(name="w1p", bufs=2) as w1p,
                tc.tile_pool(name="w2p", bufs=2) as w2p,
                tc.tile_pool(name="x2sp", bufs=4) as x2sp,
                tc.tile_pool(name="ysp", bufs=2) as ysp,
                tc.tile_pool(name="fbc", bufs=2) as fbc,
            ):
                def build_h2(fb):
                    ab_bfs, cb_bfs = [], []
                    for sub in range(fsub):
                        gsl = slice(fb * FB + sub * SB, fb * FB + (sub + 1) * SB)
                        ab_ps = bcast(a2_all[:, gsl])
                        cb_ps = bcast(c2_all[:, gsl])
                        ab_bf = fbc.tile([P, SB], BF, tag="fab", name="fab")
                        nc.scalar.activation(ab_bf[:], ab_ps[:], AF.Copy)
                        cb_bf = fbc.tile([P, SB], BF, tag="fcb", name="fcb")
                        nc.scalar.activation(cb_bf[:], cb_ps[:], AF.Copy)
                        ab_bfs.append(ab_bf)
                        cb_bfs.append(cb_bf)
                    h2 = h2p.tile([P, nkt, FB], BF, tag="h2")
                    for sub in range(fsub):
                        dsl = slice(sub * SB, (sub + 1) * SB)
                        for kc in range(nkt // KC):
                            x2t = x2sp.tile([P, KC, SB], BF, tag="x2l", bufs=3)
                            nc.gpsimd.dma_start(
                                x2t[:],
                                x2_d[fsub * fb + sub, kc * KC:(kc + 1) * KC].transpose([1, 0, 2]))
                            for j in range(KC):
                                kt = kc * KC + j
                                nc.vector.tensor_tensor(h2[:, kt, dsl], x2t[:, j, :], ab_bfs[sub][:], ALU.mult)
                                nc.vector.tensor_tensor(h2[:, kt, dsl], h2[:, kt, dsl], cb_bfs[sub][:], ALU.add)
                                if ln2_affine:
                                    nc.vector.tensor_scalar(
                                        h2[:, kt, dsl], h2[:, kt, dsl],
                                        ln2_wb[:, kt:kt + 1], ln2_wb[:, nkt + kt:nkt + kt + 1],
                                        ALU.mult, ALU.add)
                    return h2

                def u_phase(fb, h2):
                    ut = utp.tile([P, nft, FB], BF, tag="ut")
                    for ft in range(nft):
                        w1f = w1p.tile([P, nkt * P], BF, tag="w1")
                        nc.sync.dma_start(w1f[:], w1_d[ft])
                        u_pss = [psmm.tile([P, SB], FP, tag="mm", name=f"ups{s}")
                                 for s in range(fsub)]
                        for kt in range(nkt):
                            for s2 in range(fsub):
                                nc.tensor.matmul(
                                    u_pss[s2][:], w1f[:, kt * P:(kt + 1) * P],
                                    h2[:, kt, s2 * SB:(s2 + 1) * SB],
                                    start=(kt == 0), stop=(kt == nkt - 1))
                        for s2 in range(fsub):
                            nc.scalar.activation(ut[:, ft, s2 * SB:(s2 + 1) * SB],
                                                 u_pss[s2][:], AF.Silu)
                    return ut

                def y_phase(fb, ut):
                    for hc in range(nkt):
                        w2h = w2p.tile([P, nft * P], BF, tag="w2")
                        nc.sync.dma_start(w2h[:], w2_d[hc])
                        y_pss = [psmm.tile([P, SB], FP, tag="mm", name=f"yps{s}")
                                 for s in range(fsub)]
                        for ft in range(nft):
                            for s2 in range(fsub):
                                nc.tensor.matmul(
                                    y_pss[s2][:], w2h[:, ft * P:(ft + 1) * P],
                                    ut[:, ft, s2 * SB:(s2 + 1) * SB],
                                    start=(ft == 0), stop=(ft == nft - 1))
                        for s2 in range(fsub):
                            gsl = slice(fb * FB + s2 * SB, fb * FB + (s2 + 1) * SB)
                            x2t = x2sp.tile([P, SB], BF, tag="x2r")
                            nc.gpsimd.dma_start(x2t[:], x2_d[fsub * fb + s2, hc])
                            yt = ysp.tile([P, SB], FP, tag="yt")
                            nc.vector.scalar_tensor_tensor(
                                yt[:], x2t[:], 1.0 / n_cores, y_pss[s2][:],
                                ALU.mult, ALU.add)
                            nc.gpsimd.dma_start(y_d[hc, :, gsl], yt[:])

                # emission order interleaves block 1's LN-apply (vector) with
                # block 0's w2 matmuls (PE) so the PE never idles at the seam
                h2_0 = build_h2(0)
                ut_0 = u_phase(0, h2_0)
                stage_e(nsb - 2)
                stage_e(nsb - 1)
                h2_1 = build_h2(1)
                y_phase(0, ut_0)
                ut_1 = u_phase(1, h2_1)
                y_phase(1, ut_1)

    _lp.__exit__(None, None, None)
    nc.compile()
    return nc


# ---------------------------------------------------------------------------
# host side
# ---------------------------------------------------------------------------

def make_core_inputs(inputs, cfg, mask_mode, ln1_affine, ln2_affine):
    seq, hid, ffn = cfg["seq"], cfg["hid"], cfg["ffn"]
    n_cores, n_heads = cfg["n_cores"], cfg["n_heads"]
    d_nope, d_rope, d_v = cfg["d_nope"], cfg["d_rope"], cfg["d_v"]
    SB = cfg["sb"]
    half = d_rope // 2
    hpc = n_heads // n_cores
    nkt = hid // P
    nsb = seq // SB
    sbt = SB // P
    nskt = seq // P
    n_rope_ot = hpc * d_rope // P
    qo = hpc * d_nope // P + n_rope_ot
    dvc = hpc * d_v
    ndvt = dvc // P
    fpc = ffn // n_cores
    nft = fpc // P

    f32 = np.float32
    x = np.asarray(inputs["hidden_states"], dtype=f32)[0]        # [seq, hid]
    xt = np.ascontiguousarray(x.T.reshape(nkt, P, seq)).astype(BF_NP)

    inv = (1.0 / (10000.0 ** (np.arange(0, d_rope, 2, dtype=f32) / f32(d_rope)))).astype(f32)
    t = np.arange(seq, dtype=f32)
    freqs = t[:, None] * inv[None, :]
    cosT = np.cos(freqs).astype(f32).T                      # [half, seq]
    sinT = np.sin(freqs).astype(f32).T
    cos128 = np.ascontiguousarray(np.tile(cosT, (P // half, 1))).astype(BF_NP)
    sin128 = np.ascontiguousarray(
        np.tile(np.concatenate([-sinT, sinT], axis=0), (P // d_rope, 1))).astype(BF_NP)
    rperm = np.zeros((P, P), dtype=f32)
    for blk in range(P // d_rope):
        b = blk * d_rope
        for i in range(half):
            rperm[b + half + i, b + i] = 1.0
            rperm[b + i, b + half + i] = 1.0

    common = {"xt": xt, "cos_t": cos128, "sin_t": sin128,
              "rperm": rperm.astype(BF_NP),
              "onc": np.ones((P, 1), dtype=BF_NP),
              "onr": np.ones((1, P), dtype=f32)}
    mask = np.asarray(inputs["attention_mask"], dtype=f32)[0, 0]  # [seq, seq]
    if mask_mode == "causal":
        m01 = np.zeros((nsb, P, sbt * SB), dtype=f32)
        qcol = np.arange(SB)
        for qb in range(nsb):
            for i in range(sbt):
                krow = (qb * sbt + i) * P + np.arange(P)[:, None]
                m01[qb, :, i * SB:(i + 1) * SB] = (qb * SB + qcol[None, :]) >= krow
        common["mask_t"] = m01.astype(BF_NP)
    elif mask_mode == "full":
        mT = np.ascontiguousarray(mask.T)                         # [sk, sq]
        m = np.empty((nskt, nsb, P, SB), dtype=f32)
        for tt in range(nskt):
            for qb in range(nsb):
                m[tt, qb] = mT[tt * P:(tt + 1) * P, qb * SB:(qb + 1) * SB]
        common["mask_t"] = m
    if ln1_affine:
        common["ln1_wb"] = np.ascontiguousarray(np.stack(
            [np.asarray(inputs["ln1_w"], f32), np.asarray(inputs["ln1_b"], f32)]
        ).reshape(2, nkt, P).transpose(2, 0, 1).reshape(P, 2 * nkt))
    if ln2_affine:
        common["ln2_wb"] = np.ascontiguousarray(np.stack(
            [np.asarray(inputs["ln2_w"], f32), np.asarray(inputs["ln2_b"], f32)]
        ).reshape(2, nkt, P).transpose(2, 0, 1).reshape(P, 2 * nkt))

    wq = np.asarray(inputs["w_q"], f32)
    wk = np.asarray(inputs["w_k"], f32)
    wv = np.asarray(inputs["w_v"], f32)
    wo = np.asarray(inputs["w_o"], f32)
    w1 = np.asarray(inputs["w1"], f32)
    w2 = np.asarray(inputs["w2"], f32)

    def batch_ot(w_rows):
        """[n*P out rows, hid] -> [n, P, nkt*P]: tile (ot)[p, kt*P+c] =
        w_rows[ot*P + c, kt*P + p] (transposed chunks, batched per out tile)."""
        n = w_rows.shape[0] // P
        return np.ascontiguousarray(
            w_rows.reshape(n, P, nkt, P).transpose(0, 3, 2, 1)
        ).reshape(n, P, nkt * P).astype(BF_NP)

    in_maps = []
    for c in range(n_cores):
        heads = range(c * hpc, (c + 1) * hpc)
        parts = []
        for w in (wq, wk):
            nope = np.concatenate([w[g * d_nope:(g + 1) * d_nope] for g in heads])
            rope = np.concatenate(
                [w[n_heads * d_nope + g * d_rope: n_heads * d_nope + (g + 1) * d_rope]
                 for g in heads])
            parts.append(np.concatenate([nope, rope]))
        wqk_t = batch_ot(np.concatenate(parts))                   # [2*qo, P, nkt*P]
        wv_c = np.concatenate([wv[g * d_v:(g + 1) * d_v] for g in heads])  # [dvc, hid]
        wv_t = np.ascontiguousarray(wv_c.T.reshape(nkt, P, dvc)).astype(BF_NP)
        wo_c = wo[:, c * dvc:(c + 1) * dvc]                       # [hid, dvc]
        wo_t = np.ascontiguousarray(
            wo_c.reshape(nkt, P, ndvt, P).transpose(0, 3, 2, 1)
        ).reshape(nkt, P, ndvt * P).astype(BF_NP)
        w1_c = w1[c * fpc:(c + 1) * fpc]                          # [fpc, hid]
        w1_t = np.ascontiguousarray(
            w1_c.reshape(nft, P, nkt, P).transpose(0, 3, 2, 1)
        ).reshape(nft, P, nkt * P).astype(BF_NP)
        w2_c = w2[:, c * fpc:(c + 1) * fpc]                       # [hid, fpc]
        w2_t = np.ascontiguousarray(
            w2_c.reshape(nkt, P, nft, P).transpose(0, 3, 2, 1)
        ).reshape(nkt, P, nft * P).astype(BF_NP)
        in_maps.append(dict(common, wqk_t=wqk_t, wv_t=wv_t, wo_t=wo_t,
                            w1_t=w1_t, w2_t=w2_t))
    return in_maps


def detect_mask_mode(mask, seq):
    if not mask.any():
        return "zero"
    iu = np.triu_indices(seq, 1)
    upper_blocked = bool((mask[iu] <= -1e8).all())
    il = np.tril_indices(seq)
    lower_zero = bool((mask[il] == 0).all())
    if upper_blocked and lower_zero:
        return "causal"
    return "full"


_BUILT = {}


def run_layer(inputs, cfg, trace=False):
    f32 = np.float32
    mask = np.asarray(inputs["attention_mask"], dtype=f32)[0, 0]
    mask_mode = detect_mask_mode(mask, cfg["seq"])
    ln1_affine = not ((np.asarray(inputs["ln1_w"]) == 1).all()
                     and (np.asarray(inputs["ln1_b"]) == 0).all())
    ln2_affine = not ((np.asarray(inputs["ln2_w"]) == 1).all()
                     and (np.asarray(inputs["ln2_b"]) == 0).all())
    key = (tuple(sorted(cfg.items())), mask_mode, ln1_affine, ln2_affine)
    if key not in _BUILT:
        _BUILT[key] = build_layer_kernel(cfg, mask_mode, ln1_affine, ln2_affine)
    nc = _BUILT[key]
    in_maps = make_core_inputs(inputs, cfg, mask_mode, ln1_affine, ln2_affine)
    res = run_bass_kernel_spmd(nc, in_maps, core_ids=list(range(cfg["n_cores"])),
                               trace=trace)
    acc = np.zeros((cfg["hid"], cfg["seq"]), dtype=np.float64)
    for c in range(cfg["n_cores"]):
        acc += res.results[c]["y_t"].reshape(cfg["hid"], cfg["seq"])
    out = acc.T.astype(f32)[None]
    return out, res


def kernel(**inputs):
    out, _ = run_layer(inputs, CFG_FULL)
    return out

